# revision 76
# baseline (speedup 1.0000x reference)
"""Multi-head attention (B=32, T=512, E=768, H=12) on 8 trn2 NeuronCores.

Sharding: data-parallel over batch (4 batches per core). Weights replicated.
All matmuls run in bf16 (1 cycle/row on the PE vs ~1.5 measured for fp32r);
accumulation stays fp32 in PSUM. rel-err budget is 2e-2; bf16 rounding of
inputs/intermediates lands ~4e-3.

Per-core layout strategy:
  - host feeds xT [4, E, T] (pre-transposed) in bf16, W.T [E, E] for Q/K/O,
    and an augmented V weight [E, 792] with per-head column blocks of 66
    (64 data columns + 1 zero "mask" column + 1 pad). The V bias is folded
    into the output bias host-side (bo_eff = bo + wo @ bv — exact since
    softmax rows sum to 1), so V projection needs no bias add.
  - Q/K projections produce QT/KT [E, T] with head-dim on partitions; Q bias
    applied by DVE tensor_scalar, K bias by ScalarE Identity-activation
    (engine balancing), both fused with the PSUM->SBUF bf16 move.
  - V lands in natural [t, head-blocks] layout; the attention mask (0/1 per
    key) is multiplied into V rows during the PSUM->SBUF move, and written
    into each head's mask column (equivalent to -inf score masking after
    renormalization).
  - scores are computed transposed: scoresT[k, q] = sum_d K[k,d] Q[q,d].
    Per k-tile one [128, 2, 512] PSUM tile holds the even-head block (bank
    A) and odd-head block (bank B); the two matmuls are row-tile pairs
    (K=64, base partitions 0/64) executing concurrently, and ONE ScalarE
    exp (FD=1024, scale=1/sqrt(d)) converts both to bf16 probs — per-k-tile
    granularity keeps the scores->exp->ctx pipeline fine-grained.
  - ctxT[d, q] + sums row = matmul(lhsT=[V_h | mask], rhs=probsT) accumulated
    over k tiles. Normalization per head pair: two DVE copies move the two
    sums rows to partition 0 of a [1, 2, 512] tile, two accumulating rank-1
    PE broadcasts through host-fed selector rows, one DVE reciprocal, two
    DVE multiplies reading ctx straight from PSUM into bf16 mergedT.
  - output projection consumes mergedT tiles as lhsT; bias added by DVE
    from PE-broadcast bias tiles; result DMA'd out in fp32 natural layout.

Scheduling notes (load-bearing):
  - The next batch's QKV emission is interleaved into the attention head
    loop (generator "pump") so the shared pp-tag PSUM slots alternate
    between attention bps and projection groups — otherwise Tile's
    allocation-order slot assignment serializes them and the PE starves
    (HAM clock-gate oscillation, ~25% cold time).
  - PSUM budget is exactly 8 banks: proj 2 + scores 2x2 + ctx 2x1.
  - DMA queues: xt/mask/out on Sync, wq on Scalar (parallel stream at
    start), wk/wv/wo + small consts on GpSimd — dma_start costs ~0.6us of
    issuing-engine time, so weight streams live on otherwise-idle engines.
  - q/k/v/merged tiles are double-buffered across batches (bf16 halves the
    SBUF footprint vs fp32, which is what makes this fit).
"""

import numpy as np
import ml_dtypes

import concourse.bass as bass
import concourse.mybir as mybir
import concourse.tile as tile
from concourse import bacc
from concourse.bass_utils import run_bass_kernel_spmd

F32 = mybir.dt.float32
BF16 = mybir.dt.bfloat16
AF = mybir.ActivationFunctionType
ALU = mybir.AluOpType
NPBF16 = ml_dtypes.bfloat16

N_CORES = 8
B, T, E = 32, 512, 768
H, D = 12, 64
BPC = B // N_CORES          # batches per core
TT = T // 128               # token tiles per batch (4)
ET = E // 128               # embed tiles (6)
VW = H * 66                 # augmented V width (792)
VC = ((0, 462), (462, 330))  # V output chunks (7 + 5 head blocks)


def build_nc():
    nc = bacc.Bacc("TRN2", target_bir_lowering=False, num_devices=N_CORES)

    xt = nc.dram_tensor("xt", [BPC, 128, ET, T], BF16, kind="ExternalInput")
    wqt = nc.dram_tensor("wqt", [128, ET, E], BF16, kind="ExternalInput")
    wkt = nc.dram_tensor("wkt", [128, ET, E], BF16, kind="ExternalInput")
    wvta = nc.dram_tensor("wvta", [128, ET, VW], BF16, kind="ExternalInput")
    wot = nc.dram_tensor("wot", [128, ET, E], BF16, kind="ExternalInput")
    bq2 = nc.dram_tensor("bq2", [128, ET], F32, kind="ExternalInput")
    bk2 = nc.dram_tensor("bk2", [128, ET], F32, kind="ExternalInput")
    bo = nc.dram_tensor("bo", [E], BF16, kind="ExternalInput")
    maskf = nc.dram_tensor("maskf", [BPC, 128, TT], F32, kind="ExternalInput")
    sel2 = nc.dram_tensor("sel2", [2, 128], BF16, kind="ExternalInput")
    ones = nc.dram_tensor("ones", [128], BF16, kind="ExternalInput")
    out = nc.dram_tensor("out", [BPC, T, E], F32, kind="ExternalOutput")

    with tile.TileContext(nc) as tc, nc.allow_low_precision(
        "bf16 tiles feed the PE; rounding to bf16 is intentional"
    ):
        with (
            tc.tile_pool(name="consts", bufs=1) as consts,
            tc.tile_pool(name="work", bufs=1) as work,
            tc.tile_pool(name="pp", bufs=2, space="PSUM") as pp,
            tc.tile_pool(name="sc", bufs=2, space="PSUM") as sc,
            tc.tile_pool(name="cx", bufs=2, space="PSUM") as cx,
        ):
            # ---- input DMAs: xt batches 0/1 on the Sync queue first, then
            # weights on the Scalar HWDGE queue so they stream in parallel.
            def load_xt(b):
                ts = []
                for et in range(ET):
                    t = work.tile([128, T], BF16, name=f"xt{b}_{et}",
                                  tag=f"xt{et}", bufs=2)
                    nc.sync.dma_start(t[:], xt[b, :, et, :])
                    ts.append(t)
                return ts

            xt_tiles = [load_xt(0)]
            if BPC > 1:
                xt_tiles.append(load_xt(1))

            def load_weight(wname, w_dr, width, eng=None, eng2=None):
                eng = eng or nc.gpsimd
                tiles = []
                for et in range(ET):
                    t = consts.tile([128, width], BF16, name=f"{wname}{et}")
                    e = eng2 if (eng2 is not None and et >= 4) else eng
                    e.dma_start(t[:], w_dr[:, et, :])
                    tiles.append(t)
                return tiles

            bq_sb = consts.tile([128, ET], F32, name="bq_sb")
            bk_sb = consts.tile([128, ET], F32, name="bk_sb")
            nc.gpsimd.dma_start(bq_sb[:], bq2[:, :])
            nc.gpsimd.dma_start(bk_sb[:], bk2[:, :])

            selE_sb = consts.tile([1, 128], BF16, name="selE_sb")
            nc.gpsimd.dma_start(selE_sb[:], sel2[0:1, :])
            selO_sb = consts.tile([1, 128], BF16, name="selO_sb")
            nc.gpsimd.dma_start(selO_sb[:], sel2[1:2, :])
            ones_sb = consts.tile([1, 128], BF16, name="ones_sb")
            nc.gpsimd.dma_start(
                ones_sb[:], ones.rearrange("(p o) -> p o", p=1)
            )
            bo_row = consts.tile([1, E], BF16, name="bo_row")
            nc.gpsimd.dma_start(
                bo_row[:], bo.rearrange("(p o) -> p o", p=1)
            )
            ones12 = consts.tile([128, 12], BF16, name="ones12")
            nc.gpsimd.memset(ones12[:], 1.0)

            # wq on the scalar queue so it streams in parallel with xt (sync)
            # and the remaining weights (gpsimd) at kernel start
            wq_sb = load_weight("wq_sb", wqt, E, eng=nc.scalar)
            # stagger the remaining weight streams so they don't steal DMA
            # bandwidth from the critical wq/xt0 tiles at kernel start
            with tc.tile_wait_until(0.010):
                wk_sb = load_weight("wk_sb", wkt, E)
            with tc.tile_wait_until(0.016):
                wv_sb = load_weight("wv_sb", wvta, VW)
            with tc.tile_wait_until(0.022):
                wo_sb = load_weight("wo_sb", wot, E)

            # broadcast output bias row across partitions via rank-1 matmul
            bo_bc = consts.tile([128, E], F32, name="bo_bc")
            for cstart, clen in ((0, 512), (512, 256)):
                ps = pp.tile([128, 512], F32, name="bc_ps2", tag="pp")
                nc.tensor.matmul(ps[:, :clen], ones_sb[:],
                                 bo_row[:, cstart:cstart + clen],
                                 start=True, stop=True)
                nc.scalar.activation(out=bo_bc[:, cstart:cstart + clen],
                                     in_=ps[:, :clen], func=AF.Copy)

            # ---- per-batch, pipelined: QKV(b+1) emission is interleaved into
            # attention(b)'s head loop (generator) so pp-tag PSUM slots
            # alternate between attention bps and projection groups.
            qkv_state = {}

            def emit_qkv_steps(b):
                """Generator: one yield per pp-pool allocation group."""
                xt_sb = xt_tiles[b]
                if b + 2 < BPC:
                    xt_tiles.append(load_xt(b + 2))

                mk = work.tile([128, TT], F32, name="mk", bufs=2)
                nc.sync.dma_start(mk[:], maskf[b, :, :])

                qt_sb = work.tile([128, ET, T], BF16, name="qt_sb", bufs=2)
                kt_sb = work.tile([128, ET, T], BF16, name="kt_sb", bufs=2)
                v_sb = work.tile([128, TT, H, 66], BF16, name="v_sb", bufs=2)
                qkv_state[b] = (qt_sb, kt_sb, v_sb)

                for dst, w_sb, b_sb, use_act in ((qt_sb, wq_sb, bq_sb, False),
                                                 (kt_sb, wk_sb, bk_sb, True)):
                    for ot in range(ET):
                        ps = pp.tile([128, 512], F32, name="proj_ps", tag="pp")
                        for et in range(ET):
                            nc.tensor.matmul(
                                ps[:],
                                w_sb[et][:, ot * 128:(ot + 1) * 128],
                                xt_sb[et][:],
                                start=(et == 0), stop=(et == ET - 1),
                            )
                        if use_act:
                            # K bias+move on ScalarE to offload the DVE
                            nc.scalar.activation(
                                out=dst[:, ot, :], in_=ps[:], func=AF.Identity,
                                bias=b_sb[:, ot:ot + 1],
                            )
                        else:
                            nc.vector.tensor_scalar_add(
                                dst[:, ot, :], ps[:], b_sb[:, ot:ot + 1]
                            )
                        yield

                v_flat = v_sb[:].rearrange("p t h c -> p t (h c)")
                for tt in range(TT):
                    for cstart, clen in VC:
                        ps = pp.tile([128, 512], F32, name="vproj_ps", tag="pp")
                        for et in range(ET):
                            nc.tensor.matmul(
                                ps[:, :clen],
                                xt_sb[et][:, tt * 128:(tt + 1) * 128],
                                wv_sb[et][:, cstart:cstart + clen],
                                start=(et == 0), stop=(et == ET - 1),
                            )
                        nc.vector.tensor_scalar_mul(
                            v_flat[:, tt, cstart:cstart + clen],
                            ps[:, :clen], mk[:, tt:tt + 1],
                        )
                    # each head's mask column: 0 from the zero weight column,
                    # overwritten with the mask value itself
                    nc.vector.tensor_scalar_mul(
                        v_sb[:, tt, :, 64:65],
                        ones12[:].unsqueeze(2), mk[:, tt:tt + 1],
                    )
                    yield

            def emit_qkv(b):
                for _ in emit_qkv_steps(b):
                    pass

            def emit_attention(b, pump=None):
                qt_sb, kt_sb, v_sb = qkv_state.pop(b)
                merged_sb = work.tile([128, ET, T], BF16, name="merged_sb",
                                      bufs=2)

                def emit_scores(j):
                    # one [E|O] PSUM tile + one exp per k-tile
                    probs = []
                    for kt in range(TT):
                        sps = sc.tile([128, 2, 512], F32, name="sps", tag="sc")
                        ksl = slice(kt * 128, (kt + 1) * 128)
                        nc.tensor.matmul(
                            sps[:, 0, :], kt_sb[0:64, j, ksl],
                            qt_sb[0:64, j, :], start=True, stop=True,
                        )
                        nc.tensor.matmul(
                            sps[:, 1, :], kt_sb[64:128, j, ksl],
                            qt_sb[64:128, j, :], start=True, stop=True,
                        )
                        p = work.tile([128, 2, 512], BF16, name="probs",
                                      tag="probs", bufs=6)
                        nc.scalar.activation(out=p[:], in_=sps[:],
                                             func=AF.Exp, scale=0.125)
                        probs.append(p)
                    return probs

                def emit_ctx_norm(j, probs):
                    cpsE = cx.tile([66, 512], F32, name="cpsE", tag="cx")
                    cpsO = cx.tile([66, 512], F32, name="cpsO", tag="cx")
                    for kt in range(TT):
                        p = probs[kt]
                        nc.tensor.matmul(
                            cpsE[:], v_sb[:, kt, 2 * j, :],
                            p[:, 0, :], start=(kt == 0), stop=(kt == TT - 1),
                        )
                        nc.tensor.matmul(
                            cpsO[:], v_sb[:, kt, 2 * j + 1, :],
                            p[:, 1, :], start=(kt == 0), stop=(kt == TT - 1),
                        )
                    # sums rows -> [1,2,512] staging tile (partition 0), then
                    # one rank-1 broadcast matmul per half through host-fed
                    # selector rows; reciprocal + per-half normalization on
                    # the DVE reading ctx straight from PSUM.
                    rr2 = work.tile([1, 2, 512], BF16, name="rr2", tag="rr2",
                                    bufs=2)
                    nc.vector.tensor_copy(rr2[0:1, 0, :], cpsE[64:65, :])
                    nc.vector.tensor_copy(rr2[0:1, 1, :], cpsO[64:65, :])
                    bps = pp.tile([128, 512], F32, name="bps", tag="pp")
                    nc.tensor.matmul(bps[:], selE_sb[:], rr2[0:1, 0, :],
                                     start=True, stop=False)
                    nc.tensor.matmul(bps[:], selO_sb[:], rr2[0:1, 1, :],
                                     start=False, stop=True)
                    rb = work.tile([128, 512], F32, name="rb", tag="rb", bufs=2)
                    nc.vector.reciprocal_approx_fast(out=rb[:], in_=bps[:])
                    nc.vector.tensor_mul(
                        merged_sb[0:64, j, :], cpsE[0:64, :], rb[0:64, :]
                    )
                    nc.vector.tensor_mul(
                        merged_sb[64:128, j, :], cpsO[0:64, :], rb[64:128, :]
                    )

                def pump_steps(n):
                    if pump is None:
                        return
                    for _ in range(n):
                        if next(pump, StopIteration) is StopIteration:
                            return

                pending = None
                for j in range(H // 2):
                    pr = emit_scores(j)
                    if pending is not None:
                        emit_ctx_norm(*pending)
                    pump_steps(3)
                    pending = (j, pr)
                emit_ctx_norm(*pending)
                pump_steps(16)
                return merged_sb

            def emit_oproj(b, merged_sb):
                for tt in range(TT):
                    o_sb = work.tile([128, E], F32, name="o_sb",
                                     tag="o_sb", bufs=4)
                    for ci, (cstart, clen) in enumerate(((0, 512), (512, 256))):
                        ps = pp.tile([128, 512], F32, name="oproj_ps", tag="pp")
                        for mt in range(ET):
                            nc.tensor.matmul(
                                ps[:, :clen],
                                merged_sb[:, mt, tt * 128:(tt + 1) * 128],
                                wo_sb[mt][:, cstart:cstart + clen],
                                start=(mt == 0), stop=(mt == ET - 1),
                            )
                        nc.vector.tensor_add(
                            o_sb[:, cstart:cstart + clen],
                            ps[:, :clen],
                            bo_bc[:, cstart:cstart + clen],
                        )
                        # drain each chunk as soon as its bias-add lands so
                        # the final output transfer overlaps the remaining
                        # compute instead of serializing into the tail
                        eng = nc.sync if (2 * tt + ci) % 2 == 0 else nc.scalar
                        eng.dma_start(
                            out[b, tt * 128:(tt + 1) * 128,
                                cstart:cstart + clen],
                            o_sb[:, cstart:cstart + clen],
                        )

            emit_qkv(0)
            for b in range(BPC):
                pump = emit_qkv_steps(b + 1) if b + 1 < BPC else None
                merged = emit_attention(b, pump)
                emit_oproj(b, merged)

    nc.finalize()
    return nc


_NC = None


def _get_nc():
    global _NC
    if _NC is None:
        _NC = build_nc()
    return _NC


def make_in_maps(x, attention_mask, wq, bq, wk, bk, wv, bv, wo, bo):
    x = np.asarray(x, dtype=np.float32)
    attention_mask = np.asarray(attention_mask)

    def wshuf(w):
        # [o, e] -> [p, et, o] with e = et*128 + p (partition-contiguous DMA)
        return np.ascontiguousarray(
            np.asarray(w, dtype=np.float32).reshape(E, ET, 128).transpose(2, 1, 0)
        ).astype(NPBF16)

    wqt = wshuf(wq)
    wkt = wshuf(wk)
    wot = wshuf(wo)
    wvt = np.asarray(wv, dtype=np.float32).T          # [E(in), E(out)]
    bq = np.asarray(bq, dtype=np.float32)
    bk = np.asarray(bk, dtype=np.float32)
    bv = np.asarray(bv, dtype=np.float32)
    # fold the V bias into the output bias: (merged + bv) @ wo.T + bo
    # == merged @ wo.T + (bo + wo @ bv), exact because softmax rows sum to 1
    bo_eff = (
        np.asarray(bo, dtype=np.float32)
        + np.asarray(wo, dtype=np.float32) @ bv
    ).astype(NPBF16)

    # augmented V weight: per-head 66-column blocks; column 64 stays zero in
    # the weight (becomes the mask column on-device), column 65 is padding.
    wvta_flat = np.zeros((E, VW), dtype=np.float32)
    for h in range(H):
        wvta_flat[:, h * 66:h * 66 + 64] = wvt[:, h * 64:(h + 1) * 64]
    wvta = np.ascontiguousarray(
        wvta_flat.reshape(ET, 128, VW).transpose(1, 0, 2)
    ).astype(NPBF16)

    bq2 = np.ascontiguousarray(bq.reshape(ET, 128).T)
    bk2 = np.ascontiguousarray(bk.reshape(ET, 128).T)
    sel2 = np.zeros((2, 128), dtype=np.float32)
    sel2[0, 0:64] = 1.0
    sel2[1, 64:128] = 1.0
    sel2 = sel2.astype(NPBF16)
    onesv = np.ones(128, dtype=NPBF16)
    maskf_full = np.ascontiguousarray(
        np.asarray(attention_mask, dtype=np.float32)
        .reshape(B, TT, 128).transpose(0, 2, 1)
    )  # [B, 128, TT]

    in_maps = []
    for c in range(N_CORES):
        sl = slice(c * BPC, (c + 1) * BPC)
        in_maps.append({
            "xt": np.ascontiguousarray(
                x[sl].reshape(BPC, T, ET, 128).transpose(0, 3, 2, 1)
            ).astype(NPBF16),
            "maskf": np.ascontiguousarray(maskf_full[sl]),
            "wqt": wqt, "wkt": wkt, "wvta": wvta, "wot": wot,
            "bq2": bq2, "bk2": bk2, "bo": bo_eff,
            "sel2": sel2, "ones": onesv,
        })
    return in_maps


def kernel(**inputs):
    in_maps = make_in_maps(**inputs)
    res = run_bass_kernel_spmd(_get_nc(), in_maps, core_ids=list(range(N_CORES)))
    return np.concatenate([res.results[c]["out"] for c in range(N_CORES)], axis=0)
